# revision 1
# baseline (speedup 1.0000x reference)
"""Trainium2 Bass kernel for nn_DetectionBEVLoss (bf16 pipeline, v3).

Takes FULL inputs (B=8,...), shards batch across 8 NeuronCores (one batch
element per core), computes per-core partial sums of the 6 loss terms plus the
positive count on-device, and finishes the tiny reduction on host.

Key optimizations over the f32 baseline:
 - host casts inputs to bf16 and pre-arranges [P, C*S] layouts (half the DMA,
   no on-device transposes/casts; bf16 doubles DVE tensor_tensor throughput)
 - pred/target channels interleaved on host so per-quad element ops run at
   2S width (halves per-instruction fixed overhead); corner tiles likewise
   interleaved, giving a contiguous 8-way extent fold
 - rotated rects are parallelograms: opposite edges are +/-E, so each clip
   pass needs only 2 shared cross-product tensors G (not 4), and each edge
   pair forms a slab whose inside-interval is [min(U0,U2), max(U0,U2)]
 - Green's theorem: the boundary contribution of a clipped segment is
   (t1-t0)*cross(S_k, S_{k+1}); no intersection points are materialized
 - 1/C1 = C1*exp(-ln(C1^2+1e-30)) evaluated on the idle Act engine (sign
   comes out automatically, exact-0 C1 yields 0 -> no NaNs)
 - both passes' G/C1 issued before the U stages with sl1/BCE vector work
   slotted in between, hiding the Act-chain latency; focal split around the
   extent/vterm work for the same reason
 - scalar_tensor_tensor avoided in hot @4S ops (STT doesn't get bf16 2x
   packing on the DVE; plain TT does) via pre-negated cross tiles
 - alpha_c denominator computed as relu(1-iou)+v to survive bf16 rounding
"""
import numpy as np

P = 128
S = 512          # free-dim elements per channel slot (65536 px per core)
NPX = P * S
EPS = 1e-7

_CACHE = {}


def _ensure_ntff_hook():
    import sys, types
    if "antenv.axon_hooks" in sys.modules:
        return
    try:
        import trn_agent_boot.trn_boot as tb
        hook = tb._ntff_profile_via_ctypes('/opt/axon/libaxon_pjrt.so')
        mod = types.ModuleType("antenv.axon_hooks")
        mod.get_axon_ntff_profile_hook = lambda: hook
        sys.modules["antenv.axon_hooks"] = mod
    except Exception:
        pass


def _build(debug=False, lvl=99):
    import concourse.bacc as bacc
    import concourse.tile as tile
    import concourse.mybir as mybir
    import concourse.bass as bass

    F32 = mybir.dt.float32
    BF = mybir.dt.bfloat16
    U8 = mybir.dt.uint8
    Alu = mybir.AluOpType
    Act = mybir.ActivationFunctionType
    AX_X = mybir.AxisListType.X
    PI2 = float(np.pi / 2)

    nc = bacc.Bacc("TRN2", target_bir_lowering=False, debug=False, num_devices=8)

    for v in [PI2, 1e-30, 1.0]:
        t = nc.alloc_sbuf_tensor(f"const-f32-{v}", [P, 1], F32)
        nc.gpsimd.memset(t.ap(), v)
        nc.const_aps.aps[(F32, v)] = t.ap()
    nc.all_engine_barrier()

    # GEO layout (interleaved pred/target):
    #   slots: 0 cxP 1 cxT 2 cyP 3 cyT 4 lP 5 lT 6 wP 7 wT 8 yawP 9 yawT
    d_geoy = nc.dram_tensor("geoy", [P, 2 * S], BF, kind="ExternalInput")
    d_geo = nc.dram_tensor("geo", [P, 8 * S], BF, kind="ExternalInput")
    d_cls = nc.dram_tensor("cls", [P, 10 * S], BF, kind="ExternalInput")
    d_zbp = nc.dram_tensor("zbp", [P, 4 * S], BF, kind="ExternalInput")
    d_zbt = nc.dram_tensor("zbt", [P, 4 * S], BF, kind="ExternalInput")
    d_ioup = nc.dram_tensor("ioup", [P, S], BF, kind="ExternalInput")
    d_iout = nc.dram_tensor("iout", [P, S], BF, kind="ExternalInput")
    d_ctf = nc.dram_tensor("ctf", [P, S], BF, kind="ExternalInput")
    d_w = nc.dram_tensor("w", [P, S], BF, kind="ExternalInput")
    d_out = nc.dram_tensor("out", [P, 8], F32, kind="ExternalOutput")

    V = nc.vector
    A = nc.scalar
    G = nc.gpsimd

    dbg_outs = []

    def dump(name, t):
        if not debug:
            return
        shp = [t.shape[0], int(np.prod(t.shape[1:]))]
        d = nc.dram_tensor(f"dbg_{name}", shp, t.dtype, kind="ExternalOutput")
        nc.sync.dma_start(out=d[:, :], in_=t)
        dbg_outs.append(name)

    def bc(t, i, k):
        # broadcast S-slice i of tile t over k slots
        b_ = t[:, i * S:(i + 1) * S]
        return bass.AP(tensor=b_.tensor, offset=b_.offset,
                       ap=[b_.ap[0], [0, k], [1, S]])

    def strided(t, start, num, step=2):
        # [P][num][S] view of S-slots start, start+step, ... of tile t
        b_ = t[:, start * S:(start + 1) * S]
        return bass.AP(tensor=b_.tensor, offset=b_.offset,
                       ap=[b_.ap[0], [step * S, num], [1, S]])

    def sl(t, i, k=1):
        return t[:, i * S:(i + k) * S]

    with tile.TileContext(nc) as tc:
      with tc.tile_pool(name="persist", bufs=1) as pp:
        ACCS = pp.tile([P, 8], F32, name="ACCS")
        ZACC = pp.tile([P, 4], F32, name="ZACC")
        W = pp.tile([P, S], BF, name="W")
        CTF = pp.tile([P, S], BF, name="CTF")
        IOUP = pp.tile([P, S], BF, name="IOUP")
        IOUT = pp.tile([P, S], BF, name="IOUT")
        GEO = pp.tile([P, 10 * S], BF, name="GEO")
        # corner tiles, interleaved: slot 2k = quad A corner k, 2k+1 = quad B
        CORX = pp.tile([P, 8 * S], BF, name="CORX")
        CORY = pp.tile([P, 8 * S], BF, name="CORY")
        # crosses, interleaved like the corners: slot 2j+q = quad q, edge j
        CAB = pp.tile([P, 8 * S], BF, name="CAB")
        CABn = pp.tile([P, 8 * S], BF, name="CABn")
        # uv smalls, interleaved: 0 uxP 1 uxT 2 vxP 3 vxT 4 uyP 5 uyT 6 vyP 7 vyT
        UVT = pp.tile([P, 8 * S], BF, name="UVT")
        SAB = pp.tile([P, S], BF, name="SAB")
        Vv = pp.tile([P, S], BF, name="Vv")
        D2C2 = pp.tile([P, S], BF, name="D2C2")

        GY = GEO[:, 8 * S:10 * S]
        l2 = sl(GEO, 4, 2); w2 = sl(GEO, 6, 2)

        # yaw first (Sin dep), then l/w (UV-product dep), then the rest
        nc.sync.dma_start(out=GY, in_=d_geoy[:, :])
        nc.sync.dma_start(out=GEO[:, 4 * S:8 * S], in_=d_geo[:, 4 * S:8 * S])
        nc.sync.dma_start(out=W, in_=d_w[:, :])
        nc.sync.dma_start(out=GEO[:, 0:4 * S], in_=d_geo[:, 0:4 * S])
        nc.sync.dma_start(out=CTF, in_=d_ctf[:, :])
        nc.sync.dma_start(out=IOUP, in_=d_ioup[:, :])
        nc.sync.dma_start(out=IOUT, in_=d_iout[:, :])

        # ============ corners + uv smalls (both quads at 2S width) ============
        with tc.tile_pool(name="corn", bufs=1) as pc:
            co2 = pc.tile([P, 2 * S], BF, name="co2")
            si2 = pc.tile([P, 2 * S], BF, name="si2")
            A.activation(co2, GY, Act.Sin, bias=PI2)
            A.activation(si2, GY, Act.Sin)
            UX2 = sl(UVT, 0, 2); VX2 = sl(UVT, 2, 2)
            UY2 = sl(UVT, 4, 2); VY2 = sl(UVT, 6, 2)
            V.tensor_tensor(UX2, l2, co2, Alu.mult)
            V.tensor_tensor(VX2, w2, si2, Alu.mult)
            V.tensor_tensor(UY2, l2, si2, Alu.mult)
            V.tensor_tensor(VY2, w2, co2, Alu.mult)
            V.tensor_reduce(ACCS[:, 6:7], W, AX_X, Alu.add)
            As2 = pc.tile([P, 2 * S], BF, name="As2")
            Ad2 = pc.tile([P, 2 * S], BF, name="Ad2")
            Ps2 = pc.tile([P, 2 * S], BF, name="Ps2")
            Pd2 = pc.tile([P, 2 * S], BF, name="Pd2")
            V.tensor_tensor(As2, UX2, VX2, Alu.add)
            V.tensor_tensor(Ad2, UX2, VX2, Alu.subtract)
            V.tensor_tensor(Ps2, UY2, VY2, Alu.add)
            V.tensor_tensor(Pd2, UY2, VY2, Alu.subtract)
            cx2 = sl(GEO, 0, 2); cy2 = sl(GEO, 2, 2)
            V.scalar_tensor_tensor(sl(CORX, 0, 2), As2, 0.5, cx2, Alu.mult, Alu.add)
            V.scalar_tensor_tensor(sl(CORX, 2, 2), Ad2, -0.5, cx2, Alu.mult, Alu.add)
            V.scalar_tensor_tensor(sl(CORX, 4, 2), As2, -0.5, cx2, Alu.mult, Alu.add)
            V.scalar_tensor_tensor(sl(CORX, 6, 2), Ad2, 0.5, cx2, Alu.mult, Alu.add)
            V.scalar_tensor_tensor(sl(CORY, 0, 2), Pd2, 0.5, cy2, Alu.mult, Alu.add)
            V.scalar_tensor_tensor(sl(CORY, 2, 2), Ps2, -0.5, cy2, Alu.mult, Alu.add)
            V.scalar_tensor_tensor(sl(CORY, 4, 2), Pd2, -0.5, cy2, Alu.mult, Alu.add)
            V.scalar_tensor_tensor(sl(CORY, 6, 2), Ps2, 0.5, cy2, Alu.mult, Alu.add)
            dump("CORX", CORX); dump("CORY", CORY)

            # crosses CR_{q,k} = cross(S_k, S_{k+1}), both quads jointly
            T1 = pc.tile([P, 8 * S], BF, name="crT1")
            T2 = pc.tile([P, 8 * S], BF, name="crT2")
            V.tensor_tensor(T1[:, 0:6 * S], CORX[:, 0:6 * S], CORY[:, 2 * S:8 * S], Alu.mult)
            V.tensor_tensor(T1[:, 6 * S:8 * S], CORX[:, 6 * S:8 * S], CORY[:, 0:2 * S], Alu.mult)
            V.tensor_tensor(T2[:, 0:6 * S], CORY[:, 0:6 * S], CORX[:, 2 * S:8 * S], Alu.mult)
            V.tensor_tensor(T2[:, 6 * S:8 * S], CORY[:, 6 * S:8 * S], CORX[:, 0:2 * S], Alu.mult)
            V.tensor_tensor(CAB, T1, T2, Alu.subtract)
            V.tensor_tensor(CABn, T2, T1, Alu.subtract)
            dump("CAB", CAB)

        # ============ clip passes: G + C1 for both passes first ============
        # pass 0: segments A (even corner slots, crosses CA), constraints B
        # pass 1: segments B (odd slots), constraints A
        # uv slice index of (ux, vx, uy, vy) for quad q: (0+q, 2+q, 4+q, 6+q)
        # pass tuples: (corner slot parity, CS slot base, CQ parity)
        PASSES = ((0, 1), (1, 0))
        with tc.tile_pool(name="clip", bufs=1) as pcl:
            CLS = pcl.tile([P, 10 * S], BF, name="CLS")
            nc.sync.dma_start(out=CLS, in_=d_cls[:, :])
            G5s = {}
            C1s = {}
            RS2s = {}
            for pi, (sq, qq) in enumerate(PASSES):
                SX = strided(CORX, sq, 4)
                SY = strided(CORY, sq, 4)
                for pair in range(2):
                    g5 = pcl.tile([P, 5 * S], BF, name=f"G5_{pi}_{pair}")
                    Gt = g5[:, 0:4 * S]
                    T1 = pcl.tile([P, 4 * S], BF, name=f"gT1_{pi}_{pair}", tag="gT1")
                    T2 = pcl.tile([P, 4 * S], BF, name=f"gT2_{pi}_{pair}", tag="gT2")
                    if pair == 0:
                        # E0 = (-ux, -uy): G = uy*SX - ux*SY
                        V.tensor_tensor(T1, bc(UVT, 4 + qq, 4), SX, Alu.mult)
                        V.tensor_tensor(T2, bc(UVT, 0 + qq, 4), SY, Alu.mult)
                        V.tensor_tensor(Gt, T1, T2, Alu.subtract)
                    else:
                        # E1 = (-vx, +vy): G = -(vx*SY + vy*SX); store Gneg
                        V.tensor_tensor(T1, bc(UVT, 2 + qq, 4), SY, Alu.mult)
                        V.tensor_tensor(T2, bc(UVT, 6 + qq, 4), SX, Alu.mult)
                        V.tensor_tensor(Gt, T1, T2, Alu.add)
                    A.copy(g5[:, 4 * S:5 * S], g5[:, 0:S])
                    C1 = pcl.tile([P, 4 * S], BF, name=f"C1_{pi}_{pair}")
                    if pair == 0:
                        V.tensor_tensor(C1, g5[:, S:5 * S], g5[:, 0:4 * S], Alu.subtract)
                    else:
                        # G stored negated: C1_true = Gneg_k - Gneg_{k+1}
                        V.tensor_tensor(C1, g5[:, 0:4 * S], g5[:, S:5 * S], Alu.subtract)
                    G5s[(pi, pair)] = g5
                    C1s[(pi, pair)] = C1
                    # start the Act chain for this pair immediately
                    RS2 = pcl.tile([P, 4 * S], BF, name=f"RS2_{pi}_{pair}")
                    A.activation(RS2, C1, Act.Square)
                    RS2s[(pi, pair)] = RS2
                # per-pass Ln/Exp so pass pi's RECs are ready while the
                # vector engine builds pass pi+1's G/C1
                for pair in range(2):
                    A.activation(RS2s[(pi, pair)], RS2s[(pi, pair)], Act.Ln, bias=1e-30)
                for pair in range(2):
                    A.activation(RS2s[(pi, pair)], RS2s[(pi, pair)], Act.Exp, scale=-1.0)
            # focal exp rides the already-loaded exp table
            E = pp.tile([P, 10 * S], BF, name="E")
            A.activation(E, CLS, Act.Exp)

            # ---- U, slab intervals, contributions ----
            CONTRS = []
            for pi, (sq, qq) in enumerate(PASSES):
                LOHI = []
                for pair in range(2):
                    Gt = G5s[(pi, pair)][:, 0:4 * S]
                    C1 = C1s[(pi, pair)]
                    RS2 = RS2s[(pi, pair)]
                    REC = pcl.tile([P, 4 * S], BF, name=f"REC_{pi}_{pair}", tag=f"REC_{pair}")
                    V.tensor_tensor(REC, C1, RS2, Alu.mult)
                    j0, j2 = (0, 2) if pair == 0 else (1, 3)
                    T0g = pcl.tile([P, 4 * S], BF, name=f"T0g_{pi}_{pair}", tag="gT1")
                    U0 = pcl.tile([P, 4 * S], BF, name=f"U0_{pi}_{pair}", tag=f"U0_{pair}")
                    T2g = pcl.tile([P, 4 * S], BF, name=f"T2g_{pi}_{pair}", tag="gT2")
                    U2 = pcl.tile([P, 4 * S], BF, name=f"U2_{pi}_{pair}", tag=f"U2_{pair}")
                    if pair == 0:
                        # U0 = (-CQ_j0 - G)*REC ; U2 = (CQ_j2 - G)*REC
                        V.tensor_tensor(T0g, bc(CABn, 2 * j0 + qq, 4), Gt, Alu.subtract)
                        V.tensor_tensor(T2g, bc(CAB, 2 * j2 + qq, 4), Gt, Alu.subtract)
                    else:
                        # G stored negated: U0 = (Gneg - CQ_j0)*REC
                        #                   U2 = (Gneg + CQ_j2)*REC
                        V.tensor_tensor(T0g, Gt, bc(CAB, 2 * j0 + qq, 4), Alu.subtract)
                        V.tensor_tensor(T2g, Gt, bc(CAB, 2 * j2 + qq, 4), Alu.add)
                    V.tensor_tensor(U0, T0g, REC, Alu.mult)
                    V.tensor_tensor(U2, T2g, REC, Alu.mult)
                    lo = pcl.tile([P, 4 * S], BF, name=f"lo_{pi}_{pair}", tag=f"lo_{pair}")
                    hi = pcl.tile([P, 4 * S], BF, name=f"hi_{pi}_{pair}", tag=f"hi_{pair}")
                    V.tensor_tensor(lo, U0, U2, Alu.min)
                    V.tensor_tensor(hi, U0, U2, Alu.max)
                    LOHI.append((lo, hi))
                LO = LOHI[0][0]; HI = LOHI[0][1]
                V.tensor_tensor(LO, LO, LOHI[1][0], Alu.max)
                V.tensor_tensor(HI, HI, LOHI[1][1], Alu.min)
                T0 = pcl.tile([P, 4 * S], BF, name=f"T0_{pi}", tag="gT1")
                T1v = pcl.tile([P, 4 * S], BF, name=f"T1v_{pi}", tag="gT2")
                V.tensor_scalar(T0, LO, 0.0, 1.0, Alu.max, Alu.min)
                V.tensor_scalar(T1v, HI, 1.0, 0.0, Alu.min, Alu.max)
                DT = pcl.tile([P, 4 * S], BF, name=f"DT_{pi}", tag="U0_0")
                V.tensor_tensor(DT, T1v, T0, Alu.subtract)
                V.tensor_scalar(DT, DT, 0.0, None, Alu.max)
                CONTR = pcl.tile([P, 4 * S], BF, name=f"CONTR_{pi}", tag=f"CONTR_{pi}")
                V.tensor_tensor(CONTR, DT, strided(CAB, sq, 4), Alu.mult)
                CONTRS.append(CONTR)
                dump(f"CONTR_{pi}", CONTR)
            # joint fold of both passes' contributions
            FF = pcl.tile([P, 2 * S], BF, name="FF", tag="gT1")
            GGt = pcl.tile([P, 2 * S], BF, name="GGt", tag="gT2")
            V.tensor_tensor(FF, CONTRS[0][:, 0:2 * S], CONTRS[0][:, 2 * S:4 * S], Alu.add)
            V.tensor_tensor(GGt, CONTRS[1][:, 0:2 * S], CONTRS[1][:, 2 * S:4 * S], Alu.add)
            V.tensor_tensor(FF, FF, GGt, Alu.add)
            V.tensor_tensor(SAB, sl(FF, 0), sl(FF, 1), Alu.add)
            dump("SAB", SAB)
            # ---- smooth-L1 + BCE ----
            with tc.tile_pool(name="sl1", bufs=1) as ps:
                ZBP = ps.tile([P, 4 * S], BF, name="ZBP", tag="ZBP")
                ZBT = ps.tile([P, 4 * S], BF, name="ZBT", tag="ZBT")
                nc.sync.dma_start(out=ZBP, in_=d_zbp[:, :])
                nc.sync.dma_start(out=ZBT, in_=d_zbt[:, :])
                D = ps.tile([P, 4 * S], BF, name="D", tag="D")
                AD = ps.tile([P, 4 * S], BF, name="AD", tag="AD")
                V.tensor_tensor(D, ZBP, ZBT, Alu.subtract)
                A.activation(AD, D, Act.Abs)
                M = ps.tile([P, 4 * S], BF, name="M", tag="ZBP")
                MD = ps.tile([P, 4 * S], BF, name="MD", tag="ZBT")
                V.tensor_scalar(M, AD, 1.0, None, Alu.min)
                V.tensor_tensor(MD, M, AD, Alu.mult)
                M2H = ps.tile([P, 4 * S], BF, name="M2H", tag="D")
                SL1 = ps.tile([P, 4 * S], BF, name="SL1", tag="AD")
                A.activation(M2H, M, Act.Square, scale=float(np.sqrt(0.5)))
                V.tensor_tensor(SL1, MD, M2H, Alu.subtract)
                V.tensor_tensor(SL1, SL1, bc(W, 0, 4), Alu.mult)
                V.tensor_reduce(ZACC, SL1.rearrange("p (c f) -> p c f", c=4),
                                AX_X, Alu.add)
                V.tensor_copy(ACCS[:, 2:3], ZACC[:, 0:1])
                V.tensor_copy(ACCS[:, 3:4], ZACC[:, 1:2])
                V.tensor_tensor(ACCS[:, 4:5], ZACC[:, 2:3], ZACC[:, 3:4], Alu.add)
                dump("SL1", SL1)

            with tc.tile_pool(name="bce", bufs=1) as pb:
                AXb = pb.tile([P, S], BF, name="AXb")
                SP = pb.tile([P, S], BF, name="SP")
                RL = pb.tile([P, S], BF, name="RL")
                XY = pb.tile([P, S], BF, name="XY")
                A.activation(AXb, IOUP, Act.Abs)
                EB = pb.tile([P, S], BF, name="EB")
                A.activation(EB, AXb, Act.Exp, scale=-1.0)
                A.activation(SP, EB, Act.Ln, bias=1.0)
                A.activation(RL, IOUP, Act.Relu)
                V.tensor_tensor(XY, IOUP, IOUT, Alu.mult)
                V.tensor_tensor(RL, RL, XY, Alu.subtract)
                V.tensor_tensor(RL, RL, SP, Alu.add)
                V.tensor_tensor(RL, RL, W, Alu.mult)
                V.tensor_reduce(ACCS[:, 5:6], RL, AX_X, Alu.add)
                dump("BCE", RL)


        # ============ focal part 1: folds, mask-select, pt ============
        with tc.tile_pool(name="focal", bufs=1) as pf:
            IDX10 = pf.tile([P, 10 * S], BF, name="IDX10")
            for c in range(10):
                G.memset(sl(IDX10, c), float(c))
            MK10 = pf.tile([P, 10 * S], BF, name="MK10")
            V.tensor_tensor(MK10, IDX10, bc(CTF, 0, 10), Alu.is_equal)
            EM = pf.tile([P, 10 * S], BF, name="EM")
            V.tensor_tensor(EM, E, MK10, Alu.mult)
            F1 = pf.tile([P, 5 * S], BF, name="F1")
            V.tensor_tensor(F1, E[:, 0:5 * S], E[:, 5 * S:10 * S], Alu.add)
            V.tensor_tensor(F1[:, 0:2 * S], F1[:, 0:2 * S], F1[:, 2 * S:4 * S], Alu.add)
            Ssum = pf.tile([P, S], BF, name="Ssum")
            V.tensor_tensor(Ssum, sl(F1, 0), sl(F1, 1), Alu.add)
            V.tensor_tensor(Ssum, Ssum, sl(F1, 4), Alu.add)
            F2 = pf.tile([P, 5 * S], BF, name="F2", tag="IDXr")
            V.tensor_tensor(F2, EM[:, 0:5 * S], EM[:, 5 * S:10 * S], Alu.add)
            V.tensor_tensor(F2[:, 0:2 * S], F2[:, 0:2 * S], F2[:, 2 * S:4 * S], Alu.add)
            ET = pf.tile([P, S], BF, name="ET")
            V.tensor_tensor(ET, sl(F2, 0), sl(F2, 1), Alu.add)
            V.tensor_tensor(ET, ET, sl(F2, 4), Alu.add)
            Ssf = pf.tile([P, S], F32, name="Ssf")
            V.tensor_copy(Ssf, Ssum)
            RSf = pf.tile([P, S], F32, name="RSf")
            V.reciprocal_approx_fast(RSf, Ssf)
            RS = pf.tile([P, S], BF, name="RS")
            A.copy(RS, RSf)
            PT = pf.tile([P, S], BF, name="PT")
            V.tensor_tensor(PT, ET, RS, Alu.mult)
            V.tensor_scalar(PT, PT, EPS, 1.0 - EPS, Alu.max, Alu.min)
            LG = pf.tile([P, S], BF, name="LG")
            A.activation(LG, PT, Act.Ln)

            # ============ v term part 1 (both quads at 2S width) ============
            with tc.tile_pool(name="vterm", bufs=1) as pv:
                rlf = pv.tile([P, 2 * S], F32, name="rlf")
                rwf = pv.tile([P, 2 * S], F32, name="rwf")
                V.tensor_scalar(rlf, l2, EPS, None, Alu.add)
                V.reciprocal_approx_fast(rlf, rlf)
                V.tensor_scalar(rwf, w2, 1e-30, None, Alu.add)
                V.reciprocal_approx_fast(rwf, rwf)
                rl = pv.tile([P, 2 * S], BF, name="rl")
                rw = pv.tile([P, 2 * S], BF, name="rw")
                A.copy(rl, rlf)
                A.copy(rw, rwf)
                x1 = pv.tile([P, 2 * S], BF, name="x1")
                x2 = pv.tile([P, 2 * S], BF, name="x2")
                V.tensor_tensor(x1, w2, rl, Alu.mult)
                V.tensor_tensor(x2, l2, rw, Alu.mult)
                mn = pv.tile([P, 2 * S], BF, name="mn")
                V.tensor_tensor(mn, x1, x2, Alu.min)
                aa = pv.tile([P, 2 * S], BF, name="aa")
                A.activation(aa, mn, Act.Arctan)

                # ============ extents -> c2, d2 (covers the arctan) ============
                with tc.tile_pool(name="d2c2", bufs=1) as pd:
                    exts = []
                    for (CT, op, nm) in ((CORX, Alu.max, "xmax"), (CORX, Alu.min, "xmin"),
                                         (CORY, Alu.max, "ymax"), (CORY, Alu.min, "ymin")):
                        f1 = pd.tile([P, 4 * S], BF, name=f"f1_{nm}", tag="f1")
                        V.tensor_tensor(f1, CT[:, 0:4 * S], CT[:, 4 * S:8 * S], op)
                        V.tensor_tensor(f1[:, 0:2 * S], f1[:, 0:2 * S], f1[:, 2 * S:4 * S], op)
                        ex = pd.tile([P, S], BF, name=f"ext_{nm}", tag=f"ext_{nm}")
                        V.tensor_tensor(ex, sl(f1, 0), sl(f1, 1), op)
                        exts.append(ex)
                    BW = pd.tile([P, S], BF, name="BW")
                    BH = pd.tile([P, S], BF, name="BH")
                    V.tensor_tensor(BW, exts[0], exts[1], Alu.subtract)
                    V.tensor_tensor(BH, exts[2], exts[3], Alu.subtract)
                    SQW = pd.tile([P, S], BF, name="SQW", tag="f1")
                    SQH = pd.tile([P, S], BF, name="SQH", tag="sqh")
                    A.activation(SQW, BW, Act.Square)
                    A.activation(SQH, BH, Act.Square)
                    C2 = pd.tile([P, S], F32, name="C2")
                    V.scalar_tensor_tensor(C2, SQW, EPS, SQH, Alu.add, Alu.add)
                    RC2f = pd.tile([P, S], F32, name="RC2f")
                    V.reciprocal_approx_fast(RC2f, C2)
                    RC2 = pd.tile([P, S], BF, name="RC2")
                    A.copy(RC2, RC2f)
                    # d2: (cxP-cxT)^2 + (cyP-cyT)^2 via one 2S-wide pass
                    DXY = pd.tile([P, 2 * S], BF, name="DXY")
                    V.tensor_tensor(DXY, strided(GEO, 0, 2), strided(GEO, 1, 2),
                                    Alu.subtract)
                    SQ2 = pd.tile([P, 2 * S], BF, name="SQ2")
                    A.activation(SQ2, DXY, Act.Square)
                    D2 = pd.tile([P, S], BF, name="D2t")
                    V.tensor_tensor(D2, sl(SQ2, 0), sl(SQ2, 1), Alu.add)
                    V.tensor_tensor(D2C2, D2, RC2, Alu.mult)
                    dump("D2C2", D2C2)

                # ============ v term part 2 ============
                m8 = pv.tile([P, 2 * S], U8, name="m8")
                V.tensor_scalar(m8, x1, 1.0, None, Alu.is_gt)
                tt2 = pv.tile([P, 2 * S], BF, name="tt2")
                V.tensor_scalar(tt2, aa, -1.0, PI2, Alu.mult, Alu.add)
                AT = pv.tile([P, 2 * S], BF, name="AT")
                A.copy(AT, aa)
                V.copy_predicated(AT, m8, tt2)
                DV = pv.tile([P, S], BF, name="DV")
                V.tensor_tensor(DV, sl(AT, 0), sl(AT, 1), Alu.subtract)
                A.activation(Vv, DV, Act.Square, scale=float(2.0 / np.pi))
                dump("Vv", Vv)

            # ============ iou + bev assembly, focal tail interleaved ============
            with tc.tile_pool(name="asm", bufs=1) as pa:
                INTER = pa.tile([P, S], BF, name="INTER")
                A.activation(INTER, SAB, Act.Abs, scale=0.5)
                AR2 = pa.tile([P, 2 * S], BF, name="AR2")
                V.tensor_tensor(AR2, l2, w2, Alu.mult)
                UN = pa.tile([P, S], BF, name="UN")
                V.tensor_tensor(UN, sl(AR2, 0), sl(AR2, 1), Alu.add)
                V.tensor_tensor(UN, UN, INTER, Alu.subtract)
                UNf = pa.tile([P, S], F32, name="UNf")
                V.tensor_scalar(UNf, UN, EPS, None, Alu.max)
                URCf = pa.tile([P, S], F32, name="URCf")
                V.reciprocal_approx_fast(URCf, UNf)
                URC = pa.tile([P, S], BF, name="URC")
                A.copy(URC, URCf)
                # focal tail filler while URC/OMI round-trip the Act engine
                OMP = pf.tile([P, S], BF, name="OMP")
                V.tensor_scalar(OMP, PT, -1.0, 1.0, Alu.mult, Alu.add)
                MPOSF = pf.tile([P, S], BF, name="MPOSF")
                V.tensor_scalar(MPOSF, CTF, 0.0, None, Alu.is_gt)
                ALPHn = pf.tile([P, S], BF, name="ALPHn")
                # negated alpha_t: 0.5*mpos - 0.75  (cls sum negated; host flips)
                V.tensor_scalar(ALPHn, MPOSF, 0.5, -0.75, Alu.mult, Alu.add)
                IOU = pa.tile([P, S], BF, name="IOU")
                V.tensor_tensor(IOU, INTER, URC, Alu.mult)
                OMI = pa.tile([P, S], BF, name="OMI")
                A.activation(OMI, IOU, Act.Relu, scale=-1.0, bias=1.0)
                FL = pf.tile([P, S], BF, name="FL")
                V.tensor_tensor(FL, OMP, OMP, Alu.mult)
                V.tensor_tensor(FL, FL, LG, Alu.mult)
                DEN = pa.tile([P, S], F32, name="DEN")
                V.scalar_tensor_tensor(DEN, OMI, EPS, Vv, Alu.add, Alu.add)
                DRCf = pa.tile([P, S], F32, name="DRCf")
                V.reciprocal_approx_fast(DRCf, DEN)
                DRC = pa.tile([P, S], BF, name="DRC")
                A.copy(DRC, DRCf)
                V.tensor_tensor(FL, FL, ALPHn, Alu.mult)
                V.tensor_reduce(ACCS[:, 0:1], FL, AX_X, Alu.add)
                ALC = pa.tile([P, S], BF, name="ALC")
                V.tensor_tensor(ALC, Vv, DRC, Alu.mult)
                V.tensor_tensor(ALC, ALC, Vv, Alu.mult)
                LB = pa.tile([P, S], BF, name="LB")
                V.tensor_tensor(LB, OMI, D2C2, Alu.add)
                V.tensor_tensor(LB, LB, ALC, Alu.add)
                V.tensor_tensor(LB, LB, W, Alu.mult)
                V.tensor_reduce(ACCS[:, 1:2], LB, AX_X, Alu.add)
                dump("IOU", IOU)
                dump("LBW", LB)
            dump("PT", PT)

        A.memzero(ACCS[:, 7:8])
        nc.sync.dma_start(out=d_out[:, :], in_=ACCS)

    nc.compile()
    nc._dbg_outs = dbg_outs
    return nc


def _get_nc():
    if "nc" not in _CACHE:
        _ensure_ntff_hook()
        _CACHE["nc"] = _build()
    return _CACHE["nc"]


def _prep_core(cls_b, regp_b, regt_b, ioup_b, iout_b, ct_b, w_b):
    """Build one core's input map (bf16, [P, C*S] layouts) from f32 [C,H,W]."""
    import ml_dtypes
    BF = ml_dtypes.bfloat16

    def chans(x, idxs):
        # x [C,H,W] -> [P, len(idxs)*S] slot-interleaved
        sel = x[idxs].reshape(len(idxs), P, S)
        return np.ascontiguousarray(sel.transpose(1, 0, 2).reshape(P, len(idxs) * S)).astype(BF)

    def geo_interleave(xp, xt, idxs):
        # slots [cP, cT] per channel: [P, 2*len(idxs)*S]
        selp = xp[idxs].reshape(len(idxs), P, S)
        selt = xt[idxs].reshape(len(idxs), P, S)
        inter = np.stack([selp, selt], axis=1)  # [C, 2, P, S]
        return np.ascontiguousarray(
            inter.transpose(2, 0, 1, 3).reshape(P, 2 * len(idxs) * S)).astype(BF)

    return {
        "cls": chans(cls_b, list(range(10))),
        "geo": geo_interleave(regp_b, regt_b, [0, 1, 3, 4]),
        "geoy": geo_interleave(regp_b, regt_b, [6]),
        "zbp": chans(regp_b, [2, 5, 7, 8]),
        "zbt": chans(regt_b, [2, 5, 7, 8]),
        "ioup": ioup_b.reshape(P, S).astype(BF),
        "iout": iout_b.reshape(P, S).astype(BF),
        "ctf": ct_b.reshape(P, S).astype(np.float32).astype(BF),
        "w": w_b.reshape(P, S).astype(BF),
    }


def kernel(**inputs):
    from concourse.bass_utils import run_bass_kernel_spmd

    nc = _get_nc()
    cls_pred = np.asarray(inputs["cls_pred"], dtype=np.float32)
    reg_pred = np.asarray(inputs["reg_pred"], dtype=np.float32)
    iou_pred = np.asarray(inputs["iou_pred"], dtype=np.float32)
    cls_targets = np.asarray(inputs["cls_targets"], dtype=np.int32)
    reg_targets = np.asarray(inputs["reg_targets"], dtype=np.float32)
    reg_weights = np.asarray(inputs["reg_weights"], dtype=np.float32)
    iou_targets = np.asarray(inputs["iou_targets"], dtype=np.float32)

    B = cls_pred.shape[0]
    in_maps = []
    for b in range(B):
        in_maps.append(_prep_core(cls_pred[b], reg_pred[b], reg_targets[b],
                                  iou_pred[b], iou_targets[b],
                                  cls_targets[b], reg_weights[b]))
    res = run_bass_kernel_spmd(nc, in_maps, core_ids=list(range(8)))
    _CACHE["last_result"] = res
    sums = np.zeros(8, np.float64)
    for r in res.results:
        sums += r["out"].astype(np.float64).sum(axis=0)
    num_pos = max(sums[6], 1.0)
    out = np.array([sums[0], sums[1], sums[2], sums[3], sums[4], sums[5]],
                   np.float64) / num_pos
    return out.astype(np.float32)

